# revision 1
# baseline (speedup 1.0000x reference)
"""BatchGRU Trainium2 kernel: bidirectional GRU over padded ragged graph batches.

Layout (per core, 128 graphs):
  - x_pad DRAM [301, 64, 128]  (feature-major padded input; row 300 = ones;
    fill = -1e30 so segment-max and relu(x+bias) are exact at padding)
  - per direction d in {f,b}: w_h_d [301, 900] = [w_hh.T ; bias_h_row],
    w_x_d [301, 900] = [w_ih.T ; (0..0, b_ih_n)]
  - out DRAM [64, 128, 600]  (t, graph, feat; cols 0:300 fwd, 300:600 bwd)

Per step per dir, PSUM tile P [128, 2048] (4 banks):
  bank0 cols    0:300  r preact   (xg + hg + biases)
  bank1 cols  512:812  z preact
  bank2 cols 1024:1324 hn = hg_n + b_hh_n
  bank3 cols 1536:1836 xn = xg_n + b_ih_n
  transpose staging T0/T1/T2 at cols 384:512, 896:1024, 1408:1536
"""

import numpy as np
from contextlib import ExitStack

H = 300
HP = 301
LMAX = 64
BG = 128          # graphs per core
G3 = 900
NCORES = 8
KC = [(0, 128), (128, 256), (256, 301)]   # feature chunks (incl ones row)
NEG_FILL = -60000.0   # fp16-safe; relu(-60000+b)=0, never wins a max

# PSUM column offsets within the [128, 1536] fp32 tile (3 banks).
# The per-dir T bank is time-shared within a step: the xn accumulation group
# (cols 0:300) runs early, is consumed by t2, then the h-transpose staging
# (cols 0:384) reuses the bank — start=True zeroes a whole bank, so regions
# must own their bank for the lifetime of the accumulation group.
C_R, C_Z, C_HN = 0, 512, 1024
C_XN = 0                      # inside the T tile
T_OFF = [0, 128, 256]         # transpose staging inside the 1-bank T tile


def build_gru(repeats=1, loop_repeats=1, break_chain=False, h16=False):
    import concourse.bacc as bacc
    import concourse.bass as bass
    import concourse.tile as tile
    from concourse import mybir
    from concourse.masks import make_identity

    f32 = mybir.dt.float32
    f16 = mybir.dt.float16
    AF = mybir.ActivationFunctionType
    ALU = mybir.AluOpType

    nc = bacc.Bacc()
    x_pad = nc.dram_tensor("x_pad", [HP, LMAX, BG], f16, kind="ExternalInput")
    w_h = [nc.dram_tensor(f"w_h_{d}", [HP, G3], f16, kind="ExternalInput") for d in "fb"]
    w_x = [nc.dram_tensor(f"w_x_{d}", [HP, G3], f16, kind="ExternalInput") for d in "fb"]
    fbias = nc.dram_tensor("fbias", [HP, 1], f32, kind="ExternalInput")
    out = nc.dram_tensor("out", [LMAX, BG, 2 * H], f32, kind="ExternalOutput")
    out16 = nc.dram_tensor("out16", [LMAX, BG, 2 * H], f16, kind="ExternalOutput") if False else None

    def mm(ap):
        return ap

    with tile.TileContext(nc) as tc, ExitStack() as ctx:
        const = ctx.enter_context(tc.tile_pool(name="const", bufs=1))
        tmp = ctx.enter_context(tc.tile_pool(name="tmp", bufs=1))
        hpool = [ctx.enter_context(tc.tile_pool(name=f"h_{d}", bufs=3)) for d in "fb"]
        htp = [ctx.enter_context(tc.tile_pool(name=f"ht_{d}", bufs=3)) for d in "fb"]
        gp = [ctx.enter_context(tc.tile_pool(name=f"g_{d}", bufs=3)) for d in "fb"]
        pp = [ctx.enter_context(tc.tile_pool(name=f"ps_{d}", bufs=1, space="PSUM"))
              for d in "fb"]
        tpp = [ctx.enter_context(tc.tile_pool(name=f"tp_{d}", bufs=1, space="PSUM"))
               for d in "fb"]

        hdt = f16 if h16 else f32
        from contextlib import nullcontext
        loop_cm = tc.For_i(0, loop_repeats, 1) if loop_repeats > 1 else nullcontext()
        with loop_cm:
          for _rep in range(repeats):
            ident = const.tile([128, 128], f32, tag="ident", name="ident")
            make_identity(nc, ident)
            ident16 = const.tile([128, 128], f16, tag="ident16", name="ident16")
            make_identity(nc, ident16)

            # ---- load weights (per dir, per K chunk) ----
            wht = [[None] * 3 for _ in range(2)]
            wxt = [[None] * 3 for _ in range(2)]
            for d in range(2):
                for k, (c0, c1) in enumerate(KC):
                    p = c1 - c0
                    wht[d][k] = const.tile([p, G3], f16, tag=f"wh{d}{k}", name=f"wh{d}{k}")
                    nc.sync.dma_start(out=wht[d][k], in_=w_h[d][c0:c1, :])
                    wxt[d][k] = const.tile([p, G3], f16, tag=f"wx{d}{k}", name=f"wx{d}{k}")
                    nc.sync.dma_start(out=wxt[d][k], in_=w_x[d][c0:c1, :])

            # ---- prologue: load x, compute h0T (segment max), relu in place ----
            msg = [None] * 3
            h0T = [None] * 3
            for k, (c0, c1) in enumerate(KC):
                p = c1 - c0
                msg[k] = const.tile([p, LMAX, BG], f16, tag=f"msg{k}", name=f"msg{k}")
                nc.sync.dma_start(out=msg[k], in_=x_pad[c0:c1, :, :])
                fb = const.tile([p, 1], f32, tag=f"fb{k}", name=f"fb{k}")
                nc.sync.dma_start(out=fb, in_=fbias[c0:c1, :])

                # max over time: tree reduction 64 -> 32 -> ... -> 1
                m1 = tmp.tile([128, 32, BG], f16, tag="m1", name="m1")
                nc.vector.tensor_max(m1[:p, :, :], msg[k][:, 0:32, :], msg[k][:, 32:64, :])
                w = 16
                while w >= 1:
                    nc.vector.tensor_max(
                        m1[:p, 0:w, :], m1[:p, 0:w, :], m1[:p, w : 2 * w, :]
                    )
                    w //= 2
                h0T[k] = const.tile([p, BG], f16, tag=f"h0T{k}", name=f"h0T{k}")
                nc.vector.tensor_copy(out=h0T[k], in_=m1[:p, 0, :])

                # msg = relu(x + bias); padding -> relu(-60000 + b) = 0;
                # ones row stays 1 (bias row is 0). Sliced over time so only
                # the early/late t-slices gate the first scan steps; the
                # middle overlaps with the scan.
                for (ta, tb) in ((0, 8), (56, 64), (8, 56)):
                    nc.scalar.activation(out=msg[k][:, ta:tb, :],
                                         in_=msg[k][:, ta:tb, :],
                                         func=AF.Relu, bias=fb)

            # ---- h0 (non-transposed) via PE transpose of h0T ----
            h_cur = [None, None]
            hT_cur = [[None] * 3, [None] * 3]
            for d in range(2):
                T0t = tpp[d].tile([128, 512], f32, tag=f"T{d}", name=f"T{d}")
                T16 = T0t.bitcast(f16)
                h0 = hpool[d].tile([128, 304], hdt, tag=f"h{d}", name=f"h{d}")
                for k, (c0, c1) in enumerate(KC):
                    p = c1 - c0
                    # transpose h0T [p, 128] -> [128, p] into fp16 PSUM staging
                    nc.tensor.transpose(
                        out=T16[:, 256 * k : 256 * k + p],
                        in_=h0T[k],
                        identity=ident16[0:p, 0:p],
                    )
                    if k == 0:
                        nc.scalar.copy(
                            out=h0[:, c0:c1], in_=T16[:, 256 * k : 256 * k + p]
                        )
                    else:
                        nc.vector.tensor_copy(
                            out=h0[:, c0:c1], in_=T16[:, 256 * k : 256 * k + p]
                        )
                nc.vector.memset(h0[:, 300:304], 1.0)
                h_cur[d] = h0
                hT_cur[d] = list(h0T)

            # ---- main scan ----
            # Phase-structured emission: both directions' same-phase ops are
            # adjacent in each engine's (strict-FIFO) queue, so the f and b
            # recurrence chains overlap instead of serializing behind each
            # other's late-phase ops.
            for s in range(LMAX):
                ts = {0: s, 1: LMAX - 1 - s}
                Prs, Pzs, Phs, Tts = {}, {}, {}, {}
                rzs, t1s, t2s, ngs, us, ws, hns = {}, {}, {}, {}, {}, {}, {}
                for d in range(2):
                    Prs[d] = pp[d].tile([128, 512], f32, tag=f"Pr{d}", name=f"Pr{d}")
                    Pzs[d] = pp[d].tile([128, 512], f32, tag=f"Pz{d}", name=f"Pz{d}")
                    Phs[d] = pp[d].tile([128, 512], f32, tag=f"Ph{d}", name=f"Ph{d}")
                    Tts[d] = tpp[d].tile([128, 512], f32, tag=f"T{d}", name=f"T{d}")

                # ---- GEMMs ----
                for d in range(2):
                    t = ts[d]
                    for k in range(3):
                        lhsT = mm(msg[k][:, t, :])
                        nc.tensor.matmul(Prs[d][:, 0:300], lhsT,
                                         mm(wxt[d][k][:, 0:300]),
                                         start=(k == 0), stop=False)
                        nc.tensor.matmul(Pzs[d][:, 0:300], lhsT,
                                         mm(wxt[d][k][:, 300:600]),
                                         start=(k == 0), stop=False)
                    for k in range(3):
                        nc.tensor.matmul(Tts[d][:, 0:300], mm(msg[k][:, t, :]),
                                         mm(wxt[d][k][:, 600:900]),
                                         start=(k == 0), stop=(k == 2))
                    for k in range(3):
                        lhsT = mm(hT_cur[d][k])
                        nc.tensor.matmul(Prs[d][:, 0:300], lhsT,
                                         mm(wht[d][k][:, 0:300]),
                                         start=False, stop=(k == 2))
                        nc.tensor.matmul(Pzs[d][:, 0:300], lhsT,
                                         mm(wht[d][k][:, 300:600]),
                                         start=False, stop=(k == 2))
                        nc.tensor.matmul(Phs[d][:, 0:300], lhsT,
                                         mm(wht[d][k][:, 600:900]),
                                         start=(k == 0), stop=(k == 2))

                # ---- sigmoids ----
                for d in range(2):
                    rz = gp[d].tile([128, 2, 300], hdt, tag=f"rz{d}", name=f"rz{d}")
                    rzs[d] = rz
                    nc.scalar.activation(out=rz[:, 0, :], in_=Prs[d][:, 0:300],
                                         func=AF.Sigmoid)
                for d in range(2):
                    nc.scalar.activation(out=rzs[d][:, 1, :], in_=Pzs[d][:, 0:300],
                                         func=AF.Sigmoid)

                # ---- n preact ----
                for d in range(2):
                    t1s[d] = gp[d].tile([128, 300], hdt, tag=f"t1{d}", name=f"t1{d}")
                    nc.vector.tensor_mul(t1s[d], rzs[d][:, 0, :], Phs[d][:, 0:300])
                for d in range(2):
                    t2s[d] = gp[d].tile([128, 300], hdt, tag=f"t2{d}", name=f"t2{d}")
                    nc.vector.tensor_add(t2s[d], t1s[d], Tts[d][:, 0:300])
                for d in range(2):
                    ngs[d] = gp[d].tile([128, 300], hdt, tag=f"n{d}", name=f"n{d}")
                    nc.scalar.activation(out=ngs[d], in_=t2s[d], func=AF.Tanh)

                # ---- h update: h' = z*h + (1-z)*n = u - (z-1)*n ----
                for d in range(2):
                    us[d] = gp[d].tile([128, 300], hdt, tag=f"u{d}", name=f"u{d}")
                    nc.gpsimd.tensor_mul(us[d], rzs[d][:, 1, :], h_cur[d][:, 0:300])
                for d in range(2):
                    ws[d] = gp[d].tile([128, 300], hdt, tag=f"w{d}", name=f"w{d}")
                    nc.vector.scalar_tensor_tensor(
                        out=ws[d], in0=rzs[d][:, 1, :], scalar=1.0, in1=ngs[d],
                        op0=ALU.subtract, op1=ALU.mult,
                    )
                for d in range(2):
                    h_new = hpool[d].tile([128, 304], hdt, tag=f"h{d}", name=f"h{d}")
                    hns[d] = h_new
                    nc.vector.tensor_sub(h_new[:, 0:300], us[d], ws[d])
                    nc.vector.memset(h_new[:, 300:304], 1.0)
                    if h16:
                        h32 = gp[d].tile([128, 300], f32, tag=f"h32{d}",
                                         name=f"h32{d}")
                        nc.gpsimd.tensor_copy(out=h32, in_=h_new[:, 0:300])
                        nc.sync.dma_start(
                            out=out[ts[d], :, d * H : (d + 1) * H], in_=h32)
                    else:
                        nc.sync.dma_start(
                            out=out[ts[d], :, d * H : (d + 1) * H],
                            in_=h_new[:, 0:300])

                # ---- transpose h' for the next step ----
                if s < LMAX - 1:
                    for d in range(2):
                        for k, (c0, c1) in enumerate(KC):
                            p = c1 - c0
                            if h16:
                                T16v = Tts[d].bitcast(f16)
                                nc.tensor.transpose(
                                    out=T16v[0:p, 2 * T_OFF[k] : 2 * T_OFF[k] + 128],
                                    in_=hns[d][:, c0:c1],
                                    identity=ident16,
                                )
                            else:
                                nc.tensor.transpose(
                                    out=Tts[d][0:p, T_OFF[k] : T_OFF[k] + 128],
                                    in_=hns[d][:, c0:c1],
                                    identity=ident,
                                )
                    for d in range(2):
                        hTn = [None] * 3
                        for k, (c0, c1) in enumerate(KC):
                            p = c1 - c0
                            hTn[k] = htp[d].tile([p, 128], f16, tag=f"hT{d}{k}",
                                                 name=f"hT{d}{k}")
                            if h16:
                                srcap = Tts[d].bitcast(f16)[
                                    0:p, 2 * T_OFF[k] : 2 * T_OFF[k] + 128]
                            else:
                                srcap = Tts[d][0:p, T_OFF[k] : T_OFF[k] + 128]
                            if k == 1:
                                nc.scalar.copy(out=hTn[k], in_=srcap)
                            else:
                                nc.vector.tensor_copy(out=hTn[k], in_=srcap)
                        if not break_chain:
                            hT_cur[d] = hTn
                for d in range(2):
                    h_cur[d] = hns[d]

    return nc


# ---------------- host side ----------------

def prep_inputs(node, batch, pos, bias, w_ih_f, w_hh_f, b_ih_f, b_hh_f,
                w_ih_b, w_hh_b, b_ih_b, b_hh_b):
    """Build per-core in_maps for the bass kernel."""
    node = np.ascontiguousarray(np.asarray(node, dtype=np.float32))
    batch = np.asarray(batch, dtype=np.int64)
    pos = np.asarray(pos, dtype=np.int64)

    # global scatter: x_pad_all [301, NCORES, 64, 128]
    x_pad_all = np.full((HP, NCORES * LMAX * BG), NEG_FILL, dtype=np.float16)
    x_pad_all = x_pad_all.reshape(HP, NCORES, LMAX, BG)
    x_pad_all[H, :, :, :] = 1.0
    core = batch // BG
    g_loc = batch % BG
    x_pad_all[0:H, core, pos, g_loc] = node.T.astype(np.float16)
    # note: fancy index above with [0:H, core, pos, g_loc]: first dim slice +
    # three aligned index arrays -> result [300, N]; assignment takes node.T.

    def wset(w_ih, w_hh, b_ih, b_hh):
        w_h_aug = np.zeros((HP, G3), dtype=np.float32)
        w_h_aug[0:H, :] = np.asarray(w_hh, np.float32).T
        bh = np.asarray(b_hh, np.float32)
        bi = np.asarray(b_ih, np.float32)
        w_h_aug[H, 0:600] = bi[0:600] + bh[0:600]
        w_h_aug[H, 600:900] = bh[600:900]
        w_x_aug = np.zeros((HP, G3), dtype=np.float32)
        w_x_aug[0:H, :] = np.asarray(w_ih, np.float32).T
        w_x_aug[H, 600:900] = bi[600:900]
        return w_h_aug.astype(np.float16), w_x_aug.astype(np.float16)

    w_h_f_aug, w_x_f_aug = wset(w_ih_f, w_hh_f, b_ih_f, b_hh_f)
    w_h_b_aug, w_x_b_aug = wset(w_ih_b, w_hh_b, b_ih_b, b_hh_b)
    fb = np.zeros((HP, 1), dtype=np.float32)
    fb[0:H, 0] = np.asarray(bias, np.float32)

    in_maps = []
    for c in range(NCORES):
        in_maps.append({
            "x_pad": np.ascontiguousarray(x_pad_all[:, c]),
            "w_h_f": w_h_f_aug, "w_x_f": w_x_f_aug,
            "w_h_b": w_h_b_aug, "w_x_b": w_x_b_aug,
            "fbias": fb,
        })
    return in_maps, core, g_loc, pos


def gather_output(results, core, g_loc, pos):
    """results: list of per-core {'out': [64,128,600]} -> [N, 600]"""
    outs = np.stack([np.asarray(r["out"]) for r in results])  # [8, 64, 128, 600]
    return outs[core, pos, g_loc, :]


# ---------------- entry point ----------------

_CACHE = {}


def _get_nc():
    if "nc" not in _CACHE:
        nc = build_gru()
        nc.finalize()
        _CACHE["nc"] = nc
    return _CACHE["nc"]


def kernel(**inputs):
    """Full-input / full-output BatchGRU kernel distributed over 8 NeuronCores."""
    from concourse.bass_utils import run_bass_kernel_spmd

    in_maps, core, g_loc, pos = prep_inputs(
        inputs["node"], inputs["batch"], inputs["pos"], inputs["bias"],
        inputs["w_ih_f"], inputs["w_hh_f"], inputs["b_ih_f"], inputs["b_hh_f"],
        inputs["w_ih_b"], inputs["w_hh_b"], inputs["b_ih_b"], inputs["b_hh_b"],
    )
    res = run_bass_kernel_spmd(_get_nc(), in_maps, core_ids=list(range(NCORES)))
    return gather_output(res.results, core, g_loc, pos).astype(np.float32)

